# revision 5
# baseline (speedup 1.0000x reference)
"""Bass/Trainium2 kernel for nn_AlternativeSelfAttention (dense transformer), V2.

Shapes: N=4, S=1024, E=1024, H=16, D=64.  8 NeuronCores.

Sharding (hardcoded): core c handles batch n = c//2 and query rows
[ (c%2)*512 , (c%2)*512+512 ) of that batch, for ALL 16 heads.  No
collectives; each core writes a disjoint [512, 1024] slice of the output.

Math (per core, per head h):
    A   = Wq.T @ Wk                      (64x64, tiny)
    Qp  = Xq_h @ A                       (so E_h = Qp_h @ Xk_h.T == q @ k.T)
    P   = exp(E_h / 32)                  (no max-subtraction; |E/32| < ~1.5)
    C_h = P_h @ Xv_h ; denom = P_h.sum(k)   (denom via 64 ones-columns in the
                                             PV stationary -> replicated rows)
    O_h = (C_h / denom) @ Wv.T
    out = concat_h(O_h) @ Wu.T + bu

V2 vs V1: all input casts f32->bf16 happen inside the SWDGE load DMAs; all
transposes (xq, xk, wu) run on the DMA xbar (SBUF->SBUF) instead of the
TensorEngine; the xv ones-interleave runs in 4x-mode DVE copies from bf16
with the big ones-memset on the idle GpSimd engine; the unifyheads bias is
injected as a rank-1 matmul at the head of each final-projection PSUM chain.
"""

import sys

sys.path.insert(0, "/opt/trn_rl_repo")

import numpy as np

import concourse.bass as bass
import concourse.mybir as mybir
import concourse.tile as tile
from concourse import bacc
from concourse.bass_utils import run_bass_kernel_spmd

F32 = mybir.dt.float32
BF16 = mybir.dt.bfloat16
AF = mybir.ActivationFunctionType
ALU = mybir.AluOpType

S = 1024          # keys/values sequence length
Q = 512           # queries per core
E = 1024          # embed
H = 16            # heads
D = 64            # head dim
KC = S // 128     # 8 key chunks
EC = E // 128     # 8 embed chunks
QC = Q // 128     # 4 query-row chunks
SCALE = 1.0 / 32.0  # 1/sqrt(E)

# energy jobs: k-chunks grouped 3/3/2 so one job = 3 PSUM banks and the
# PSUM budget (2x3 energy + 2 small rotating) fits exactly.
JOB_CHUNKS = ((0, 1, 2), (3, 4, 5), (6, 7))
CHUNK2JOB = {c: ((c // 3, c % 3) if c < 6 else (2, c - 6)) for c in range(KC)}


def _body(nc, tc, xq, xk, xv, wq, wk, wv, wu, bu, idin, out):
    with (
        tc.tile_pool(name="pp", bufs=1) as pp,
        tc.tile_pool(name="ptp", bufs=8) as ptp,
        tc.tile_pool(name="cnp", bufs=2) as cnp,
        tc.tile_pool(name="ep", bufs=2, space="PSUM") as ep,
        tc.tile_pool(name="cp", bufs=2, space="PSUM") as cp,
    ):
        # ---------------- constants / small loads ----------------
        ident_f = pp.tile([128, 128], F32)
        nc.sync.dma_start(ident_f[:], idin)
        zbias = pp.tile([128, 1], F32)
        nc.vector.memset(zbias[:], 0.0)
        ones_f = pp.tile([1, 128], F32)   # bias-broadcast matmul stationary
        nc.vector.memset(ones_f[:], 1.0)

        wq_s = pp.tile([D, D], F32)
        nc.sync.dma_start(wq_s[:], wq)
        wk_s = pp.tile([D, D], F32)
        nc.sync.dma_start(wk_s[:], wk)
        wv_s = pp.tile([D, D], F32)
        nc.sync.dma_start(wv_s[:], wv)

        # A = Wq.T @ Wk -> blkdiag(A, A) bf16
        apsum = cp.tile([D, D], F32, tag="cpt", name="apsum")
        nc.tensor.matmul(apsum[:], wq_s[:], wk_s[:])
        blkA = pp.tile([128, 128], BF16)
        nc.vector.memset(blkA[:], 0.0)
        nc.vector.tensor_copy(blkA[0:D, 0:D], apsum[:])
        nc.vector.tensor_copy(blkA[D:128, D:128], apsum[:])

        # Wv.T -> blkdiag(Wv.T, Wv.T) bf16
        wvt_ps = cp.tile([D, D], F32, tag="cpt", name="wvt_ps")
        nc.tensor.transpose(wvt_ps[:], wv_s[:], ident_f[0:D, 0:D])
        blkWvT = pp.tile([128, 128], BF16)
        nc.vector.memset(blkWvT[:], 0.0)
        nc.vector.tensor_copy(blkWvT[0:D, 0:D], wvt_ps[:])
        nc.vector.tensor_copy(blkWvT[D:128, D:128], wvt_ps[:])

        # bu replicated to all partitions via a stride-0 source DMA (HWDGE)
        bu_rep = pp.tile([128, E], F32)
        bu_bcast = bass.AP(bu.tensor, bu.offset, [[0, 128], [1, E]])
        nc.sync.dma_start(bu_rep[:], bu_bcast)

        # xv1 holds v interleaved with ones-columns: [k, chunk, head, 64on+64v].
        xv1 = pp.tile([128, KC, H * 128], BF16)
        xv1_v = xv1[:].rearrange("p j (h c) -> p j h c", c=128)

        # ---------------- cast loads + xbar transposes ----------------
        # All nat-chunk staging tiles live in co-resident pools (no address
        # reuse between tensors) so the SWDGE load stream has no false
        # dependencies on earlier tensors' transposes.  xq first: the Qp path
        # is the longest dependency chain before energy.  xk transposes
        # dispatch on the scalar HWDGE queue (idle until the first exp);
        # xq/wu transposes + stores keep the sync queue.
        xqT = pp.tile([128, EC, Q], BF16)    # [e, q]
        xkT = pp.tile([128, EC, S], BF16)    # [e, k]
        wuT = pp.tile([128, EC, E], BF16)    # [e, e']
        with (
            tc.tile_pool(name="natk", bufs=4) as natk,
            tc.tile_pool(name="natb", bufs=3) as natb,
        ):
            xqn = natb.tile([128, 4, E], BF16, tag="natb", name="xqn")
            nc.gpsimd.dma_start(
                xqn[:], xq.rearrange("(j p) e -> p j e", p=128)
            )
            for j in range(QC):
                nc.sync.dma_start(
                    xqT[:, :, j * 128 : (j + 1) * 128],
                    xqn[:, j, :],
                    transpose=True,
                )

            xkns = []
            for jj in range(4):
                xkn = natk.tile([128, 2, E], BF16, tag="natk", name=f"xkn{jj}")
                nc.gpsimd.dma_start(
                    xkn[:],
                    xk[jj * 256 : (jj + 1) * 256, :].rearrange(
                        "(j p) e -> p j e", p=128
                    ),
                )
                xkns.append(xkn)
                for jc in range(2):
                    j = 2 * jj + jc
                    nc.scalar.dma_start(
                        xkT[:, :, j * 128 : (j + 1) * 128],
                        xkn[:, jc, :],
                        transpose=True,
                    )

            xvns = []
            for jj in range(2):
                xvn = natb.tile([128, 4, E], BF16, tag="natb", name=f"xvn{jj}")
                nc.gpsimd.dma_start(
                    xvn[:],
                    xv[jj * 512 : (jj + 1) * 512, :].rearrange(
                        "(j p) e -> p j e", p=128
                    ),
                )
                xvns.append(xvn)

            wuns = []
            for jj in range(2):
                wun = natb.tile([128, 4, E], BF16, tag="natb", name=f"wun{jj}")
                nc.gpsimd.dma_start(
                    wun[:],
                    wu[jj * 512 : (jj + 1) * 512, :].rearrange(
                        "(j p) e -> p j e", p=128
                    ),
                )
                wuns.append(wun)

            # ones in cols 0:D so the PV denominator rows land at partitions
            # 0:63 (the custom-DVE reciprocal mis-reads PSUM at a nonzero base
            # partition).  Dispatched on GpSimd AFTER the SWDGE loads so the
            # memset doesn't block the load stream.
            nc.gpsimd.memset(xv1_v[:, :, :, 0:D], 1.0)

            # Qp.T = blkdiag(A,A).T @ Xq.T per e-chunk
            qpT = pp.tile([128, EC, Q], BF16)    # [e', q]
            for t in range(EC):
                qpp = cp.tile([128, Q], F32, tag="cpt", name=f"qpp{t}")
                nc.tensor.matmul(qpp[:], blkA[:], xqT[:, t, :])
                nc.vector.tensor_copy(qpT[:, t, :], qpp[:])

            # values: 4x-mode DVE scatter-copies into the ones-interleaved xv1
            for jj in range(2):
                for jc in range(4):
                    j = 4 * jj + jc
                    nc.vector.tensor_copy(
                        xv1_v[:, j, :, D:128],
                        xvns[jj][:, jc, :].rearrange("p (h d) -> p h d", d=D),
                    )

            # Wu.T via xbar transposes (sync queue; wu loads land last)
            for jj in range(2):
                for jc in range(4):
                    j = 4 * jj + jc
                    nc.sync.dma_start(
                        wuT[:, :, j * 128 : (j + 1) * 128],
                        wuns[jj][:, jc, :],
                        transpose=True,
                    )

        # ---------------- main loop over head pairs ----------------
        oT = pp.tile([128, EC, Q], BF16)    # context.T  [e, q]
        stage = pp.tile([128, QC, E], F32)

        for p in range(8):  # pair p = heads (2p, 2p+1)
            pts = {}
            for ji, chunks in enumerate(JOB_CHUNKS):
                w = 512 * len(chunks)
                ets = []
                for hh in range(2):
                    et = ep.tile([128, w], F32, tag="et", name=f"et{2*p+hh}_{ji}")
                    ets.append(et)
                # interleave the two heads' MMs: adjacent row-groups (0-63 /
                # 64-127) execute concurrently in the PE array.
                for ci, c in enumerate(chunks):
                    for hh in range(2):
                        b0 = hh * D
                        nc.tensor.matmul(
                            ets[hh][:, ci * 512 : (ci + 1) * 512],
                            xkT[b0 : b0 + D, p, c * 128 : (c + 1) * 128],
                            qpT[b0 : b0 + D, p, :],
                        )
                for hh in range(2):
                    pt = ptp.tile([128, w], BF16, tag="pt", name=f"pt{2*p+hh}_{ji}")
                    nc.scalar.activation(
                        pt[:], ets[hh][:], AF.Exp, bias=zbias[:], scale=SCALE
                    )
                    pts[(hh, ji)] = pt

            cns = cnp.tile([128, Q], BF16, tag="cnt", name=f"cn{p}")
            for hh in range(2):
                h = 2 * p + hh
                b0 = hh * D
                cpt = cp.tile([128, Q], F32, tag="cpt", name=f"cpt{h}")
                for c in range(KC):
                    ji, ci = CHUNK2JOB[c]
                    # rows 0:64 accumulate the softmax denominator (ones
                    # columns, replicated); rows 64:128 accumulate P @ Xv_h.
                    nc.tensor.matmul(
                        cpt[:],
                        xv1_v[:, c, h, :],
                        pts[(hh, ji)][:, ci * 512 : (ci + 1) * 512],
                        start=(c == 0),
                        stop=(c == KC - 1),
                    )
                dn = cnp.tile([D, Q], F32, tag="dn", name=f"dn{h}")
                nc.vector.reciprocal_approx_fast(out=dn[:], in_=cpt[0:D, :])
                nc.vector.tensor_tensor(
                    cns[b0 : b0 + D, :], cpt[D:128, :], dn[:], op=ALU.mult
                )

            # O_pair.T = blkdiag(Wv,Wv) @ Cn_pair.T
            opt_ = cp.tile([128, Q], F32, tag="cpt", name=f"opt{p}")
            nc.tensor.matmul(opt_[:], blkWvT[:], cns[:])
            nc.vector.tensor_copy(oT[:, p, :], opt_[:])

            # final-projection batch A: 2 (s,half)-groups per pair for pairs
            # 3..6, each accumulating bias + contributions of pairs 0..p.
            if 3 <= p <= 6:
                for g in (2 * (p - 3), 2 * (p - 3) + 1):
                    s, half = divmod(g, 2)
                    fp = cp.tile([128, 512], F32, tag="cpt", name=f"fa{g}")
                    nc.tensor.matmul(
                        fp[:],
                        ones_f[:],
                        bu_rep[0:1, half * 512 : (half + 1) * 512],
                        start=True,
                        stop=False,
                    )
                    for pp_ in range(p + 1):
                        nc.tensor.matmul(
                            fp[:],
                            oT[:, pp_, s * 128 : (s + 1) * 128],
                            wuT[:, pp_, half * 512 : (half + 1) * 512],
                            start=False,
                            stop=(pp_ == p),
                        )
                    nc.vector.tensor_copy(
                        stage[:, s, half * 512 : (half + 1) * 512], fp[:]
                    )

        # final-projection batch B: remaining pair contributions, then store
        for s in range(QC):
            for half in range(2):
                g = 2 * s + half
                p_done = 3 + g // 2  # pairs 0..p_done were folded in batch A
                fp = cp.tile([128, 512], F32, tag="cpt", name=f"fb{g}")
                for pp_ in range(p_done + 1, 8):
                    nc.tensor.matmul(
                        fp[:],
                        oT[:, pp_, s * 128 : (s + 1) * 128],
                        wuT[:, pp_, half * 512 : (half + 1) * 512],
                        start=(pp_ == p_done + 1),
                        stop=(pp_ == 7),
                    )
                dst = stage[:, s, half * 512 : (half + 1) * 512]
                nc.vector.tensor_tensor(dst, dst, fp[:], op=ALU.add)
            nc.sync.dma_start(out[s * 128 : (s + 1) * 128, :], stage[:, s, :])


def build():
    nc = bacc.Bacc("TRN2", target_bir_lowering=False, debug=False)
    xq = nc.dram_tensor("xq", [Q, E], F32, kind="ExternalInput").ap()
    xk = nc.dram_tensor("xk", [S, E], F32, kind="ExternalInput").ap()
    xv = nc.dram_tensor("xv", [S, E], F32, kind="ExternalInput").ap()
    wq = nc.dram_tensor("wq", [D, D], F32, kind="ExternalInput").ap()
    wk = nc.dram_tensor("wk", [D, D], F32, kind="ExternalInput").ap()
    wv = nc.dram_tensor("wv", [D, D], F32, kind="ExternalInput").ap()
    wu = nc.dram_tensor("wu", [E, E], F32, kind="ExternalInput").ap()
    bu = nc.dram_tensor("bu", [E], F32, kind="ExternalInput").ap()
    idin = nc.dram_tensor("idin", [128, 128], F32, kind="ExternalInput").ap()
    out = nc.dram_tensor("out", [Q, E], F32, kind="ExternalOutput").ap()

    with tile.TileContext(nc) as tc:
        _body(nc, tc, xq, xk, xv, wq, wk, wv, wu, bu, idin, out)
    nc.compile()
    return nc


_NC_CACHE = []


def _get_nc():
    if not _NC_CACHE:
        _NC_CACHE.append(build())
    return _NC_CACHE[0]


def _in_maps(values, keys, query, Wk, Wq, Wv, Wu, bu):
    values = np.ascontiguousarray(np.asarray(values, dtype=np.float32))
    keys = np.ascontiguousarray(np.asarray(keys, dtype=np.float32))
    query = np.ascontiguousarray(np.asarray(query, dtype=np.float32))
    Wk = np.ascontiguousarray(np.asarray(Wk, dtype=np.float32))
    Wq = np.ascontiguousarray(np.asarray(Wq, dtype=np.float32))
    Wv = np.ascontiguousarray(np.asarray(Wv, dtype=np.float32))
    Wu = np.ascontiguousarray(np.asarray(Wu, dtype=np.float32))
    bu = np.ascontiguousarray(np.asarray(bu, dtype=np.float32))

    ident_np = np.eye(128, dtype=np.float32)
    maps = []
    for c in range(8):
        n, qh = divmod(c, 2)
        maps.append(
            {
                "xq": np.ascontiguousarray(query[n, qh * Q : (qh + 1) * Q, :]),
                "xk": keys[n],
                "xv": values[n],
                "wq": Wq,
                "wk": Wk,
                "wv": Wv,
                "wu": Wu,
                "bu": bu,
                "idin": ident_np,
            }
        )
    return maps


def _ensure_ntff_hook():
    """The agent image's antenv lacks axon_hooks; bass_utils imports it when
    trace=True.  Inject the module and install the boot's ctypes-based hook."""
    import sys as _sys
    import types as _types

    if "antenv.axon_hooks" in _sys.modules:
        return
    try:
        import antenv  # noqa: F401

        mod = _types.ModuleType("antenv.axon_hooks")
        mod._hook = None

        def set_axon_ntff_profile_hook(h):
            mod._hook = h

        def get_axon_ntff_profile_hook():
            return mod._hook

        mod.set_axon_ntff_profile_hook = set_axon_ntff_profile_hook
        mod.get_axon_ntff_profile_hook = get_axon_ntff_profile_hook
        _sys.modules["antenv.axon_hooks"] = mod
        import antenv as _ae

        _ae.axon_hooks = mod
        from trn_agent_boot.trn_boot import _ntff_profile_via_ctypes

        mod._hook = _ntff_profile_via_ctypes("/opt/axon/libaxon_pjrt.so")
    except Exception:
        pass


def run(values, keys, query, mask, Wk, Wq, Wv, Wu, bu, trace=False):
    """Returns (full_output [4,1024,1024] f32, BassKernelResults)."""
    if trace:
        _ensure_ntff_hook()
    nc = _get_nc()
    maps = _in_maps(values, keys, query, Wk, Wq, Wv, Wu, bu)
    res = run_bass_kernel_spmd(nc, maps, core_ids=list(range(8)), trace=trace)
    out = np.empty((4, S, E), dtype=np.float32)
    for c in range(8):
        n, qh = divmod(c, 2)
        out[n, qh * Q : (qh + 1) * Q, :] = res.results[c]["out"]
    return out, res


def kernel(values, keys, query, mask, Wk, Wq, Wv, Wu, bu):
    out, _ = run(values, keys, query, mask, Wk, Wq, Wv, Wu, bu, trace=False)
    return out
